# revision 1
# baseline (speedup 1.0000x reference)
"""FullAttention (non-standard multiplicative causal mask) on 8 TRN2 cores.

Reference (per batch b, head h):
    S = Q @ K^T                      [L, L]
    S = S * tril(ones)               (multiplicative mask: zeros above diag)
    A = softmax(S / sqrt(E))         (masked slots contribute exp(0)=1)
    O = A @ V

Key observation: for s > l, P[l,s] = exp(0) = 1, so
    num[l]   = sum_{s<=l} exp(z_ls) v_s  +  sum_{s>l} v_s
    denom[l] = sum_{s<=l} exp(z_ls)      +  (L-1-l)

Sharding: B*H = 32 (b,h) pairs -> 4 per core (2 "packs" of 2 heads).
Host pre-transposes Q,K to [e, l] layout (and casts to fp16) so no
on-chip transposes are needed; 2 heads are stacked on the 128 SBUF
partitions so QK^T matmuls (contraction E=64) can row-pack the PE
array via tile_position.

Per (b,h), chunk-outer flash-style loop over l-chunks of 512:
  - QK: S^T pieces [s_tile=128, l_chunk=512] via matmul(lhsT=kT, rhs=qT)
  - exp on ScalarE (PSUM->SBUF, fp16 out), causal fixup via gpsimd
    affine_select / memset(1.0)
  - PV: out^T [66, 512] += V1[s_tile].T @ P^T piece  (V1 = [V | 1 | 0]
    adds the softmax denominator as column 64); suffix (s>l)
    contributions enter as a rank-1 matmul of the V-suffix-sum
    (tilesums on PE, cascaded on VectorE).
  - finalize: PE-transpose to [l, 66], reciprocal of col 64, scale, DMA.
"""

import numpy as np

import concourse.bass as bass
import concourse.mybir as mybir
import concourse.tile as tile
from concourse import bacc
from concourse.masks import make_identity

F32 = mybir.dt.float32
F16 = mybir.dt.float16
AF = mybir.ActivationFunctionType

B, L, H, E = 2, 2048, 16, 64
D = 64
SCALE = 0.125          # 1/sqrt(64)
NCORES = 8
BH_PER_CORE = (B * H) // NCORES   # 4
PACKS = BH_PER_CORE // 2          # 2
NT = L // 128                     # 16 s-tiles
NJ = L // 512                     # 4 l-chunks
VW = 66                           # [V | 1 | 0pad] (even moving width)

# "cascade": V-suffix sums via cascaded PSUM accumulation + rank-1 matmul.
# "dense":   all-ones P^T tiles for fully-masked s_tiles (simpler, more PE).
SUFFIX_MODE = "cascade"

_cached = None


def _build_program():
    nc = bacc.Bacc("TRN2", target_bir_lowering=False)
    qt = nc.dram_tensor("qt", [PACKS, 128, L], F16, kind="ExternalInput")
    kt = nc.dram_tensor("kt", [PACKS, 128, L], F16, kind="ExternalInput")
    v1d = nc.dram_tensor("v1", [BH_PER_CORE, 128, NT * VW], F16, kind="ExternalInput")
    o = nc.dram_tensor("o", [BH_PER_CORE, L, D], F32, kind="ExternalOutput")

    with tile.TileContext(nc) as tc:
        with (
            tc.tile_pool(name="consts", bufs=1) as consts,
            tc.tile_pool(name="qk_sb", bufs=2) as qk_sb,
            tc.tile_pool(name="v1_sb", bufs=2) as v1_pool,
            tc.tile_pool(name="sufc", bufs=2) as sufc_pool,
            tc.tile_pool(name="pt", bufs=20) as pt_pool,
            tc.tile_pool(name="ot_sb", bufs=4) as ot_sb_pool,
            tc.tile_pool(name="osb", bufs=3) as o_pool,
            tc.tile_pool(name="rcp", bufs=8) as rcp_pool,
            tc.tile_pool(name="piece", bufs=3, space="PSUM") as piece_ps,
            tc.tile_pool(name="otr_ps", bufs=3, space="PSUM") as otr_ps,
            tc.tile_pool(name="small_ps", bufs=2, space="PSUM") as small_ps,
            tc.tile_pool(name="dram", bufs=2, space="DRAM") as dram_pool,
        ):
            warm_sb = consts.tile([128, 512], F16)
            nc.vector.memset(warm_sb, 0.25)
            warm_ps = piece_ps.tile([128, 512], F32, tag="pp", name="warm")
            for w in range(10):
                nc.tensor.matmul(
                    warm_ps, warm_sb[:, 0:128], warm_sb,
                    start=True, stop=True, skip_group_check=True,
                )
            ident = consts.tile([128, 128], F16)
            make_identity(nc, ident)
            ones_row = consts.tile([1, 512], F16)
            nc.vector.memset(ones_row, 1.0)
            ones_col = consts.tile([128, 1], F16)
            nc.vector.memset(ones_col, 1.0)
            ones128 = consts.tile([128, 128], F16)
            nc.vector.memset(ones128, 1.0)
            # strict lower-tri mask (1 where p > f): marks s>l inside the
            # diagonal 128x128 block of an S^T piece (uint8 for predication)
            maskl = consts.tile([128, 128], mybir.dt.int8)
            nc.gpsimd.memset(maskl, 0)
            nc.gpsimd.affine_select(
                out=maskl, in_=maskl,
                compare_op=mybir.AluOpType.is_ge, fill=1,
                base=0, pattern=[[1, 128]], channel_multiplier=-1,
            )
            if SUFFIX_MODE == "dense":
                ones_tile = consts.tile([128, 512], F16)
                nc.vector.memset(ones_tile, 1.0)

            pack_tiles = {}

            def load_pack(p):
                qt_t = qk_sb.tile([128, L], F16, tag="qt", name="qt_t")
                kt_t = qk_sb.tile([128, L], F16, tag="kt", name="kt_t")
                nc.sync.dma_start(out=qt_t, in_=qt[p])
                nc.scalar.dma_start(out=kt_t, in_=kt[p])
                v1l = []
                for hh2 in range(2):
                    v1_t = v1_pool.tile([128, NT, VW], F16, tag="v1", name="v1_t")
                    eng = nc.sync if hh2 == 0 else nc.scalar
                    eng.dma_start(
                        out=v1_t.rearrange("p t w -> p (t w)"),
                        in_=v1d[2 * p + hh2],
                    )
                    v1l.append(v1_t)
                sufbl = []
                for hh2 in range(2):
                    v1_t = v1l[hh2]
                    # tile-granular suffix table: SUF[t] = sum of [V|1|0]
                    # rows over s_tiles > t; col 64 = masked-slot count.
                    ts_sb = sufc_pool.tile([1, NT * VW], F32, tag="tssb")
                    for g in range(4):
                        sf_ps = small_ps.tile([1, 4 * VW], F32, tag="sps")
                        nc.tensor.matmul(
                            sf_ps, ones_col, v1_t[:, 4 * g:4 * g + 4, :],
                            start=True, stop=True, skip_group_check=True,
                        )
                        nc.vector.tensor_copy(
                            ts_sb[0:1, 4 * g * VW:(4 * g + 4) * VW], sf_ps
                        )
                    suf_sb = sufc_pool.tile([1, NT * VW], F32, tag="sufsb")
                    nc.vector.memset(suf_sb[0:1, (NT - 1) * VW:NT * VW], 0.0)
                    for t in range(NT - 2, -1, -1):
                        nc.vector.tensor_add(
                            suf_sb[0:1, t * VW:(t + 1) * VW],
                            suf_sb[0:1, (t + 1) * VW:(t + 2) * VW],
                            ts_sb[0:1, (t + 1) * VW:(t + 2) * VW],
                        )
                    # broadcast across partitions for the finalize-side add
                    # (SBUF APs need nonzero partition step -> bounce via DRAM)
                    suf_dr = dram_pool.tile([1, NT * VW], F32, tag="sufdr")
                    nc.sync.dma_start(out=suf_dr, in_=suf_sb)
                    sufb = sufc_pool.tile([128, NT, VW], F32, tag="sufb")
                    nc.sync.dma_start(
                        out=sufb.rearrange("p t w -> p (t w)"),
                        in_=bass.AP(tensor=suf_dr.tensor, offset=suf_dr.offset,
                                    ap=[[0, 128], [1, NT * VW]]),
                    )
                    sufbl.append(sufb)
                pack_tiles[p] = (qt_t, kt_t, v1l, sufbl)

            load_pack(0)
            for pack in range(PACKS):
                qt_t, kt_t, v1, sufc = pack_tiles.pop(pack)

                ostg = []     # per head: [128, NT, 64] output staging
                for hh in range(2):
                    ostg.append(o_pool.tile([128, NT, 64], F32, tag="osb",
                                            name="ostg"))

                for j in range(NJ):
                    if j == 2 and pack + 1 < PACKS:
                        load_pack(pack + 1)
                    lo = 512 * j
                    nk = 4 * j + 4          # s_tiles participating causally
                    ot = []
                    for hh in range(2):
                        ot_t = otr_ps.tile([VW, 512], F32, tag="otr", name="ot")
                        ot.append(ot_t)

                    pv_started = [False] * 2
                    pending_pv = []

                    def emit_pv(hh, k, pt_t):
                        is_last = (k == nk - 1) and SUFFIX_MODE == "cascade"
                        poff = 128 * max(0, k - 4 * j)
                        nc.tensor.matmul(
                            ot[hh][:, poff:512], v1[hh][:, k, :],
                            pt_t[:, poff:512],
                            start=not pv_started[hh],
                            stop=is_last,
                            skip_group_check=True,
                        )
                        pv_started[hh] = True

                    for k in range(nk):             # s_tile pieces
                        pps, pts = [], []
                        for hh in range(2):
                            pps.append(piece_ps.tile([128, 512], F32, tag="pp", name="pp"))
                            pts.append(pt_pool.tile([128, 512], F16, tag="pt", name="pt"))
                        # QK (diag pieces skip the fully-masked leading cols)
                        qoff = 128 * max(0, k - 4 * j)
                        for hh in range(2):
                            r0 = 64 * hh
                            nc.tensor.matmul(
                                pps[hh][:, qoff:512],
                                kt_t[r0:r0 + 64, 128 * k:128 * (k + 1)],
                                qt_t[r0:r0 + 64, lo + qoff:lo + 512],
                                start=True, stop=True,
                            )
                        # delayed PV from 2 pieces ago keeps PE fed
                        while len(pending_pv) > 8:
                            pending_pv.pop(0)()
                        # exp + causal fixups (fixups on DVE, off PE's path)
                        m = k - 4 * j
                        for hh in range(2):
                            pp, pt_t = pps[hh], pts[hh]
                            if m < 0:                  # plain piece
                                nc.scalar.activation(pt_t, pp, AF.Exp, scale=SCALE)
                            else:                      # diagonal piece
                                nc.scalar.activation(
                                    pt_t[:, 128 * m:512], pp[:, 128 * m:512],
                                    AF.Exp, scale=SCALE,
                                )
                                # triangle: keep where f >= p else 1.0
                                nc.gpsimd.affine_select(
                                    out=pt_t[:, 128 * m:128 * m + 128],
                                    in_=pt_t[:, 128 * m:128 * m + 128],
                                    compare_op=mybir.AluOpType.is_ge,
                                    fill=1.0,
                                    base=0,
                                    pattern=[[1, 128]],
                                    channel_multiplier=-1,
                                )
                            pending_pv.append(
                                lambda hh=hh, k=k, p=pts[hh]: emit_pv(hh, k, p)
                            )
                    for fn in pending_pv:
                        fn()
                    pending_pv = []

                    if SUFFIX_MODE == "dense":
                        for hh in range(2):
                            for k in range(nk, NT):
                                nc.tensor.matmul(
                                    ot[hh], v1[hh][:, k, :], ones_tile,
                                    start=False, stop=(k == NT - 1),
                                    skip_group_check=True,
                                )

                    # finalize chunk j: transpose (fp16), divide into staging
                    for hh in range(2):
                        bh = 2 * pack + hh
                        ot_s = ot_sb_pool.tile([VW, 512], F16, tag="ots")
                        nc.vector.tensor_copy(ot_s, ot[hh])
                        for t in range(4):
                            tg = 4 * j + t
                            tr = otr_ps.tile([128, VW], F16, tag="otr", name="tr")
                            nc.tensor.transpose(
                                tr, ot_s[:, 128 * t:128 * (t + 1)],
                                ident[0:VW, 0:VW],
                            )
                            trs = ot_sb_pool.tile([128, VW], F32, tag="trs")
                            nc.vector.tensor_add(trs, tr, sufc[hh][:, tg, :])
                            rcp = rcp_pool.tile([128, 1], F32, tag="rcp")
                            nc.vector.reciprocal(rcp, trs[:, 64:65])
                            nc.vector.tensor_scalar_mul(
                                ostg[hh][:, tg, :], trs[:, 0:64], rcp
                            )
                        if t == 3:
                            nc.scalar.dma_start(
                                out=o[bh, 512 * j:512 * (j + 1), :]
                                .rearrange("(t p) d -> p t d", p=128),
                                in_=ostg[hh][:, 4 * j:4 * (j + 1), :],
                            )

    nc.compile()
    return nc


def _get_program():
    global _cached
    if _cached is None:
        _cached = _build_program()
    return _cached


def _shard_inputs(queries, keys, values):
    # [B, L, H, E] -> [B, H, E, L] -> [BH, E, L]
    qT = np.ascontiguousarray(queries.transpose(0, 2, 3, 1)).reshape(B * H, E, L)
    kT = np.ascontiguousarray(keys.transpose(0, 2, 3, 1)).reshape(B * H, E, L)
    # [B, L, H, D] -> [BH, L, D]
    vv = np.ascontiguousarray(values.transpose(0, 2, 1, 3)).reshape(B * H, L, D)
    in_maps = []
    for c in range(NCORES):
        s = c * BH_PER_CORE
        qp = qT[s:s + BH_PER_CORE].reshape(PACKS, 128, L)
        kp = kT[s:s + BH_PER_CORE].reshape(PACKS, 128, L)
        vb = vv[s:s + BH_PER_CORE].reshape(BH_PER_CORE, NT, 128, D)
        v1h = np.zeros((BH_PER_CORE, 128, NT, VW), dtype=np.float16)
        v1h[:, :, :, 0:64] = vb.transpose(0, 2, 1, 3)
        v1h[:, :, :, 64] = 1.0
        in_maps.append({
            "qt": np.ascontiguousarray(qp).astype(np.float16),
            "kt": np.ascontiguousarray(kp).astype(np.float16),
            "v1": v1h.reshape(BH_PER_CORE, 128, NT * VW),
        })
    return in_maps


def _gather_outputs(results):
    full = np.concatenate([r["o"] for r in results], axis=0)  # [BH, L, D]
    return np.ascontiguousarray(
        full.reshape(B, H, L, D).transpose(0, 2, 1, 3)
    )  # [B, L, H, D]


def kernel(queries, keys, values, _trace=[False]):
    from concourse.bass_utils import run_bass_kernel_spmd

    queries = np.asarray(queries, dtype=np.float32)
    keys = np.asarray(keys, dtype=np.float32)
    values = np.asarray(values, dtype=np.float32)
    nc = _get_program()
    in_maps = _shard_inputs(queries, keys, values)
    res = run_bass_kernel_spmd(
        nc, in_maps, core_ids=list(range(NCORES)), trace=_trace[0]
    )
    out = _gather_outputs(res.results)
    if _trace[0]:
        kernel.last_results = res
    return out



# revision 5
# speedup vs baseline: 1.1372x; 1.1372x over previous
"""FullAttention (non-standard multiplicative causal mask) on 8 TRN2 cores.

Reference (per batch b, head h):
    S = Q @ K^T                      [L, L]
    S = S * tril(ones)               (multiplicative mask: zeros above diag)
    A = softmax(S / sqrt(E))         (masked slots contribute exp(0)=1)
    O = A @ V

Key observation: for s > l, P[l,s] = exp(0) = 1, so
    num[l]   = sum_{s<=l} exp(z_ls) v_s  +  sum_{s>l} v_s
    denom[l] = sum_{s<=l} exp(z_ls)      +  (L-1-l)

Sharding: B*H = 32 (b,h) pairs -> 4 per core (2 "packs" of 2 heads).
Host pre-transposes Q,K to [e, l] layout (and casts to fp16) so no
on-chip transposes are needed; 2 heads are stacked on the 128 SBUF
partitions so QK^T matmuls (contraction E=64) can row-pack the PE
array via tile_position.

Per (b,h), chunk-outer flash-style loop over l-chunks of 512:
  - QK: S^T pieces [s_tile=128, l_chunk=512] via matmul(lhsT=kT, rhs=qT)
  - exp on ScalarE (PSUM->SBUF, fp16 out), causal fixup via gpsimd
    affine_select / memset(1.0)
  - PV: out^T [66, 512] += V1[s_tile].T @ P^T piece  (V1 = [V | 1 | 0]
    adds the softmax denominator as column 64); suffix (s>l)
    contributions enter as a rank-1 matmul of the V-suffix-sum
    (tilesums on PE, cascaded on VectorE).
  - finalize: PE-transpose to [l, 66], reciprocal of col 64, scale, DMA.
"""

import numpy as np

import concourse.bass as bass
import concourse.mybir as mybir
import concourse.tile as tile
from concourse import bacc
from concourse.masks import make_identity

F32 = mybir.dt.float32
F16 = mybir.dt.bfloat16
AF = mybir.ActivationFunctionType

B, L, H, E = 2, 2048, 16, 64
D = 64
SCALE = 0.125          # 1/sqrt(64)
NCORES = 8
BH_PER_CORE = (B * H) // NCORES   # 4
PACKS = BH_PER_CORE // 2          # 2
NT = L // 128                     # 16 s-tiles
NJ = L // 512                     # 4 l-chunks
VW = 66                           # [V | 1 | 0pad] (even moving width)

# "cascade": V-suffix sums via cascaded PSUM accumulation + rank-1 matmul.
# "dense":   all-ones P^T tiles for fully-masked s_tiles (simpler, more PE).
SUFFIX_MODE = "cascade"

_cached = None


def _build_program():
    nc = bacc.Bacc("TRN2", target_bir_lowering=False)
    qt = nc.dram_tensor("qt", [PACKS, 128, L], F16, kind="ExternalInput")
    kt = nc.dram_tensor("kt", [PACKS, 128, L], F16, kind="ExternalInput")
    v1d = nc.dram_tensor("v1", [BH_PER_CORE, 128, NT * VW], F16, kind="ExternalInput")
    o = nc.dram_tensor("o", [BH_PER_CORE, L, D], F32, kind="ExternalOutput")

    with tile.TileContext(nc) as tc:
        with (
            tc.tile_pool(name="consts", bufs=1) as consts,
            tc.tile_pool(name="qk_sb", bufs=2) as qk_sb,
            tc.tile_pool(name="v1_sb", bufs=2) as v1_pool,
            tc.tile_pool(name="sufc", bufs=2) as sufc_pool,
            tc.tile_pool(name="pt", bufs=20) as pt_pool,
            tc.tile_pool(name="ot_sb", bufs=4) as ot_sb_pool,
            tc.tile_pool(name="osb", bufs=3) as o_pool,
            tc.tile_pool(name="rcp", bufs=8) as rcp_pool,
            tc.tile_pool(name="piece", bufs=3, space="PSUM") as piece_ps,
            tc.tile_pool(name="otr_ps", bufs=3, space="PSUM") as otr_ps,
            tc.tile_pool(name="small_ps", bufs=2, space="PSUM") as small_ps,
            tc.tile_pool(name="dram", bufs=2, space="DRAM") as dram_pool,
        ):
            warm_sb = consts.tile([128, 512], F16)
            nc.vector.memset(warm_sb, 0.25)
            warm_ps = piece_ps.tile([128, 512], F32, tag="pp", name="warm")
            for w in range(10):
                nc.tensor.matmul(
                    warm_ps, warm_sb[:, 0:128], warm_sb,
                    start=True, stop=True, skip_group_check=True,
                )
            ident = consts.tile([128, 128], F16)
            make_identity(nc, ident)
            ones_row = consts.tile([1, 512], F16)
            nc.vector.memset(ones_row, 1.0)
            ones_col = consts.tile([128, 1], F16)
            nc.vector.memset(ones_col, 1.0)
            ones128 = consts.tile([128, 128], F16)
            nc.vector.memset(ones128, 1.0)
            # strict lower-tri mask (1 where p > f): marks s>l inside the
            # diagonal 128x128 block of an S^T piece (uint8 for predication)
            maskl = consts.tile([128, 128], mybir.dt.int8)
            nc.gpsimd.memset(maskl, 0)
            nc.gpsimd.affine_select(
                out=maskl, in_=maskl,
                compare_op=mybir.AluOpType.is_ge, fill=1,
                base=0, pattern=[[1, 128]], channel_multiplier=-1,
            )
            if SUFFIX_MODE == "dense":
                ones_tile = consts.tile([128, 512], F16)
                nc.vector.memset(ones_tile, 1.0)

            pack_tiles = {}

            def load_pack(p):
                qt_t = qk_sb.tile([128, L], F16, tag="qt", name="qt_t")
                kt_t = qk_sb.tile([128, L], F16, tag="kt", name="kt_t")
                nc.sync.dma_start(out=qt_t, in_=qt[p])
                nc.gpsimd.dma_start(out=kt_t, in_=kt[p])
                v1l = []
                for hh2 in range(2):
                    v1_t = v1_pool.tile([128, NT, VW], F16, tag="v1", name="v1_t")
                    eng = nc.sync if hh2 == 0 else nc.gpsimd
                    eng.dma_start(
                        out=v1_t.rearrange("p t w -> p (t w)"),
                        in_=v1d[2 * p + hh2],
                    )
                    v1l.append(v1_t)
                sufbl = []
                for hh2 in range(2):
                    v1_t = v1l[hh2]
                    # tile-granular suffix table: SUF[t] = sum of [V|1|0]
                    # rows over s_tiles > t; col 64 = masked-slot count.
                    ts_sb = sufc_pool.tile([1, NT * VW], F32, tag="tssb")
                    for g in range(4):
                        sf_ps = small_ps.tile([1, 4 * VW], F32, tag="sps")
                        nc.tensor.matmul(
                            sf_ps, ones_col, v1_t[:, 4 * g:4 * g + 4, :],
                            start=True, stop=True, skip_group_check=True,
                        )
                        nc.vector.tensor_copy(
                            ts_sb[0:1, 4 * g * VW:(4 * g + 4) * VW], sf_ps
                        )
                    suf_sb = sufc_pool.tile([1, NT * VW], F32, tag="sufsb")
                    nc.vector.memset(suf_sb[0:1, (NT - 1) * VW:NT * VW], 0.0)
                    for t in range(NT - 2, -1, -1):
                        nc.vector.tensor_add(
                            suf_sb[0:1, t * VW:(t + 1) * VW],
                            suf_sb[0:1, (t + 1) * VW:(t + 2) * VW],
                            ts_sb[0:1, (t + 1) * VW:(t + 2) * VW],
                        )
                    # broadcast across partitions for the finalize-side add
                    # (SBUF APs need nonzero partition step -> bounce via DRAM)
                    suf_dr = dram_pool.tile([1, NT * VW], F32, tag="sufdr")
                    nc.sync.dma_start(out=suf_dr, in_=suf_sb)
                    sufb = sufc_pool.tile([128, NT, VW], F32, tag="sufb")
                    nc.sync.dma_start(
                        out=sufb.rearrange("p t w -> p (t w)"),
                        in_=bass.AP(tensor=suf_dr.tensor, offset=suf_dr.offset,
                                    ap=[[0, 128], [1, NT * VW]]),
                    )
                    sufbl.append(sufb)
                pack_tiles[p] = (qt_t, kt_t, v1l, sufbl)

            load_pack(0)
            for pack in range(PACKS):
                qt_t, kt_t, v1, sufc = pack_tiles.pop(pack)

                ostg = []     # per head: [128, NT, 64] output staging
                for hh in range(2):
                    ostg.append(o_pool.tile([128, NT, 64], F32, tag="osb",
                                            name="ostg"))

                for j in range(NJ):
                    if j == 2 and pack + 1 < PACKS:
                        load_pack(pack + 1)
                    lo = 512 * j
                    nk = 4 * j + 4          # s_tiles participating causally
                    ot = []
                    for hh in range(2):
                        ot_t = otr_ps.tile([VW, 512], F32, tag="otr", name="ot")
                        ot.append(ot_t)

                    pv_started = [False] * 2
                    pending_pv = []

                    def emit_pv(hh, k, pt_t):
                        is_last = (k == nk - 1) and SUFFIX_MODE == "cascade"
                        poff = 128 * max(0, k - 4 * j)
                        nc.tensor.matmul(
                            ot[hh][:, poff:512], v1[hh][:, k, :],
                            pt_t[:, poff:512],
                            start=not pv_started[hh],
                            stop=is_last,
                            skip_group_check=True,
                        )
                        pv_started[hh] = True

                    for k in range(nk):             # s_tile pieces
                        pps, pts = [], []
                        for hh in range(2):
                            pps.append(piece_ps.tile([128, 512], F32, tag="pp", name="pp"))
                            pts.append(pt_pool.tile([128, 512], F16, tag="pt", name="pt"))
                        # QK (diag pieces skip the fully-masked leading cols)
                        qoff = 128 * max(0, k - 4 * j)
                        for hh in range(2):
                            r0 = 64 * hh
                            nc.tensor.matmul(
                                pps[hh][:, qoff:512],
                                kt_t[r0:r0 + 64, 128 * k:128 * (k + 1)],
                                qt_t[r0:r0 + 64, lo + qoff:lo + 512],
                                start=True, stop=True,
                            )
                        # delayed PV from 2 pieces ago keeps PE fed
                        while len(pending_pv) > 8:
                            pending_pv.pop(0)()
                        # exp + causal fixups (fixups on DVE, off PE's path)
                        m = k - 4 * j
                        for hh in range(2):
                            pp, pt_t = pps[hh], pts[hh]
                            if m < 0:                  # plain piece
                                nc.scalar.activation(pt_t, pp, AF.Exp, scale=SCALE)
                            else:                      # diagonal piece
                                nc.scalar.activation(
                                    pt_t[:, 128 * m:512], pp[:, 128 * m:512],
                                    AF.Exp, scale=SCALE,
                                )
                                # triangle: keep where f >= p else 1.0
                                nc.gpsimd.affine_select(
                                    out=pt_t[:, 128 * m:128 * m + 128],
                                    in_=pt_t[:, 128 * m:128 * m + 128],
                                    compare_op=mybir.AluOpType.is_ge,
                                    fill=1.0,
                                    base=0,
                                    pattern=[[1, 128]],
                                    channel_multiplier=-1,
                                )
                            pending_pv.append(
                                lambda hh=hh, k=k, p=pts[hh]: emit_pv(hh, k, p)
                            )
                    for fn in pending_pv:
                        fn()
                    pending_pv = []

                    if SUFFIX_MODE == "dense":
                        for hh in range(2):
                            for k in range(nk, NT):
                                nc.tensor.matmul(
                                    ot[hh], v1[hh][:, k, :], ones_tile,
                                    start=False, stop=(k == NT - 1),
                                    skip_group_check=True,
                                )

                    # finalize chunk j: transpose (fp16), divide into staging
                    for hh in range(2):
                        bh = 2 * pack + hh
                        ot_s = ot_sb_pool.tile([VW, 512], F16, tag="ots")
                        nc.vector.tensor_copy(ot_s, ot[hh])
                        for t in range(4):
                            tg = 4 * j + t
                            tr = otr_ps.tile([128, VW], F16, tag="otr", name="tr")
                            nc.tensor.transpose(
                                tr, ot_s[:, 128 * t:128 * (t + 1)],
                                ident[0:VW, 0:VW],
                            )
                            trs = ot_sb_pool.tile([128, VW], F32, tag="trs")
                            nc.vector.tensor_add(trs, tr, sufc[hh][:, tg, :])
                            rcp = rcp_pool.tile([128, 1], F32, tag="rcp")
                            nc.vector.reciprocal(rcp, trs[:, 64:65])
                            nc.vector.tensor_scalar_mul(
                                ostg[hh][:, tg, :], trs[:, 0:64], rcp
                            )
                        if t == 3:
                            nc.gpsimd.dma_start(
                                out=o[bh, 512 * j:512 * (j + 1), :]
                                .rearrange("(t p) d -> p t d", p=128),
                                in_=ostg[hh][:, 4 * j:4 * (j + 1), :],
                            )

    nc.compile()
    return nc


def _get_program():
    global _cached
    if _cached is None:
        _cached = _build_program()
    return _cached


def _shard_inputs(queries, keys, values):
    import ml_dtypes
    BF16 = ml_dtypes.bfloat16
    # [B, L, H, E] -> [B, H, E, L] -> [BH, E, L]
    qT = np.ascontiguousarray(queries.transpose(0, 2, 3, 1)).reshape(B * H, E, L)
    kT = np.ascontiguousarray(keys.transpose(0, 2, 3, 1)).reshape(B * H, E, L)
    # [B, L, H, D] -> [BH, L, D]
    vv = np.ascontiguousarray(values.transpose(0, 2, 1, 3)).reshape(B * H, L, D)
    in_maps = []
    for c in range(NCORES):
        s = c * BH_PER_CORE
        qp = qT[s:s + BH_PER_CORE].reshape(PACKS, 128, L)
        kp = kT[s:s + BH_PER_CORE].reshape(PACKS, 128, L)
        vb = vv[s:s + BH_PER_CORE].reshape(BH_PER_CORE, NT, 128, D)
        v1h = np.zeros((BH_PER_CORE, 128, NT, VW), dtype=BF16)
        v1h[:, :, :, 0:64] = vb.transpose(0, 2, 1, 3)
        v1h[:, :, :, 64] = 1.0
        in_maps.append({
            "qt": np.ascontiguousarray(qp).astype(BF16),
            "kt": np.ascontiguousarray(kp).astype(BF16),
            "v1": v1h.reshape(BH_PER_CORE, 128, NT * VW),
        })
    return in_maps


def _gather_outputs(results):
    full = np.concatenate([r["o"] for r in results], axis=0)  # [BH, L, D]
    return np.ascontiguousarray(
        full.reshape(B, H, L, D).transpose(0, 2, 1, 3)
    )  # [B, L, H, D]


def kernel(queries, keys, values, _trace=[False]):
    from concourse.bass_utils import run_bass_kernel_spmd

    queries = np.asarray(queries, dtype=np.float32)
    keys = np.asarray(keys, dtype=np.float32)
    values = np.asarray(values, dtype=np.float32)
    nc = _get_program()
    in_maps = _shard_inputs(queries, keys, values)
    res = run_bass_kernel_spmd(
        nc, in_maps, core_ids=list(range(NCORES)), trace=_trace[0]
    )
    out = _gather_outputs(res.results)
    if _trace[0]:
        kernel.last_results = res
    return out

